# revision 24
# baseline (speedup 1.0000x reference)
"""PointPillarsScatter Trainium2 kernel (fp16, bandwidth-optimized).

Reference op:
  canvas[b*NY*NX + y*NX + x] = voxel_features[p]        (scatter-set, 64 ch)
  out[:, :64]  = canvas -> [B, 64, NY, NX]
  out[:, 64:]  = transpose(map_fm, (0, 3, 2, 1))        (16 ch)

Strategy (8 NeuronCores, SPMD), core = batch*2 + y_half:
  The op is pure data movement, so the kernel is sized by HBM traffic.
  Everything runs in fp16 (abs rel err ~2^-11, far inside the 2e-2
  gate): the 34MB/core of fp32 output becomes 17MB, and all input
  tables are fp16 too.

  Scatter = one-hot matmul on the TensorEngine:
    out[128ch', 512cells] = featT[K, 128ch'].T @ S[K, 512]
  with S[s, n] = (pos[s] == n) built by DVE is_equal against a
  DMA-loaded iota table.  ch' packs the 64 channels of TWO 512-cell
  tiles (tile j -> psum 0:64, tile j+105 -> 64:128) so one matmul
  covers 1024 cells and each partition half maps to contiguous DRAM.
  K (points per column) is the per-group max point count over cores,
  baked into the program, so the feat table DMA only reads the
  occupied slots (~2.2MB/core instead of a padded 12MB).

  map_fm is transposed to [CM, NYH, NX] fp16 on the host; the device
  moves it with a single DRAM->DRAM DMA on the (otherwise idle)
  gpsimd SWDGE queue.  Canvas A/B-half writes go on the sync/scalar
  HWDGE queues respectively, balancing all three DMA paths.

Host side only computes index tables + shards/reformats inputs (per
the sharding hint); all arithmetic that produces output values runs
on device.  The host up-casts the returned fp16 shards to fp32.
"""

import sys

for _p in ("/opt/trn_rl_repo",):
    if _p not in sys.path:
        sys.path.insert(0, _p)

import numpy as np

# problem constants (hardcoded per contract)
B, NPTS, C, NY, NX, CM = 4, 48000, 64, 496, 432, 16
NYH = NY // 2            # 248 rows per core
NCORE = 8
NCELL = NYH * NX         # 107136 cells per core
TILE = 512               # cells per channel-block
NT = (NCELL + TILE - 1) // TILE          # 210 tiles (last has 128 cells)
NP = NT // 2                             # 105 pairs: tile 2j with 2j+1
LAST = NCELL - (NT - 1) * TILE           # 128 cells in the last tile
CAP = 128                # max points per column (K dim of the matmul)
SG = 15                  # pairs per group -> 7 groups, ~2MB out DMAs
NGRP = (NP + SG - 1) // SG
FB = 18                  # column budget of the per-group feat tile

_prog_cache = {}


def _build_program(ncols, chunks, kg):
    """Build the SPMD Bass program (identical for all 8 cores)."""
    from concourse import bacc, mybir, tile

    f16 = mybir.dt.float16
    f32 = mybir.dt.float32

    nc = bacc.Bacc(trn_type="TRN2", target_bir_lowering=False)

    feat_d = nc.dram_tensor("feat", [CAP, ncols * 128], f16,
                            kind="ExternalInput")
    iota_d = nc.dram_tensor("iota", [CAP, TILE], f16, kind="ExternalInput")
    post_d = nc.dram_tensor("post", [CAP, ncols], f32, kind="ExternalInput")
    map_d = nc.dram_tensor("mapin", [CM, NCELL], f16, kind="ExternalInput")
    # canvas in psum layout: row = blk*64+ch, col = pair*512+pos; the
    # host de-interleaves tile halves during the gather step
    out_d = nc.dram_tensor("out", [128, NP * TILE], f16,
                           kind="ExternalOutput")
    outm_d = nc.dram_tensor("outm", [CM, NCELL], f16, kind="ExternalOutput")

    colbase = np.concatenate([[0], np.cumsum(chunks)]).astype(np.int64)

    with tile.TileContext(nc) as tc:
        with (
            tc.tile_pool(name="const", bufs=1) as cpool,
            tc.tile_pool(name="fpool", bufs=3) as fpool,
            tc.tile_pool(name="spool", bufs=4) as spool,
            tc.tile_pool(name="stg", bufs=3) as stpool,
            tc.tile_pool(name="pscat", bufs=6, space="PSUM") as pspool,
        ):
            # DMA-loaded iota table + per-column slot positions
            iota_f = cpool.tile([CAP, TILE], f16)
            nc.sync.dma_start(out=iota_f[:], in_=iota_d[:])
            posT = cpool.tile([CAP, ncols], f32)
            nc.sync.dma_start(out=posT[:], in_=post_d[:])

            # map passthrough: DRAM->DRAM chunks, one per group, so the
            # map bandwidth fills the gaps of the scatter pipeline
            # instead of starving its head.
            mchunk = (NCELL + NGRP - 1) // NGRP

            def emit_map_chunk(g, eng):
                m0 = g * mchunk
                m1 = min(m0 + mchunk, NCELL)
                eng.dma_start(out=outm_d[:, m0:m1],
                              in_=map_d[:, m0:m1])

            def emit_feat_load(g):
                p0, p1 = g * SG, min((g + 1) * SG, NP)
                c0, c1 = int(colbase[p0]), int(colbase[p1])
                K = int(kg[g])
                fb = fpool.tile([CAP, FB * 128], f16, tag="fb")
                # sync queue, emitted before older groups' out DMAs so
                # the in-order sync engine issues the prefetch first.
                # Full 128 partitions: partial-partition DMAs serialize
                # onto a single SDMA engine (~27GB/s).
                nc.sync.dma_start(out=fb[:, :(c1 - c0) * 128],
                                  in_=feat_d[:, c0 * 128:c1 * 128])
                return fb

            fbs = {0: emit_feat_load(0), 1: emit_feat_load(1)}
            for g in range(NGRP):
                p0 = g * SG
                p1 = min(p0 + SG, NP)
                c0 = int(colbase[p0])
                K = int(kg[g])
                fb = fbs.pop(g)
                if g + 2 < NGRP:
                    fbs[g + 2] = emit_feat_load(g + 2)
                emit_map_chunk(g, nc.scalar)
                stg = stpool.tile([128, SG * TILE], f16, tag="stg")
                for pr in range(p0, p1):
                    ps = pspool.tile([128, TILE], f32)
                    nck = int(chunks[pr])
                    for k in range(nck):
                        col = int(colbase[pr]) + k
                        s_t = spool.tile([CAP, TILE], f16)
                        nc.vector.tensor_scalar(
                            out=s_t[:K], in0=iota_f[:K, :],
                            scalar1=posT[:K, col:col + 1],
                            scalar2=None,
                            op0=mybir.AluOpType.is_equal)
                        lhs = fb[:K, (col - c0) * 128:(col - c0 + 1) * 128]
                        nc.tensor.matmul(
                            out=ps[:], lhsT=lhs, rhs=s_t[:K],
                            start=(k == 0), stop=(k == nck - 1))
                    off = (pr - p0) * TILE
                    if pr % 4 == 0:
                        nc.vector.tensor_copy(out=stg[:, off:off + TILE],
                                              in_=ps[:])
                    else:
                        nc.scalar.copy(out=stg[:, off:off + TILE], in_=ps[:])
                # one full-width 2MB DMA per group in psum layout
                oeng = nc.sync if g % 2 == 0 else nc.scalar
                wa = (p1 - p0) * TILE
                oeng.dma_start(out=out_d[:, p0 * TILE:p0 * TILE + wa],
                               in_=stg[:, :wa])

    nc.finalize()
    return nc


def _host_prep(voxel_features, coords, map_fm):
    """Shard points by core, build feat/pos tables (host index work only)."""
    vf = np.asarray(voxel_features).astype(np.float16)
    cd = np.asarray(coords)
    mf = np.asarray(map_fm)
    if mf.ndim == 5:
        mf = np.squeeze(mf, 3)

    b = cd[:, 0].astype(np.int64)
    y = cd[:, 2].astype(np.int64)
    x = cd[:, 3].astype(np.int64)
    valid = (b >= 0) & (b < B) & (y >= 0) & (y < NY) & (x >= 0) & (x < NX)
    b, y, x = b[valid], y[valid], x[valid]
    vfv = vf[valid]

    half = (y >= NYH).astype(np.int64)
    core = b * 2 + half
    lcell = (y - half * NYH) * NX + x
    t = lcell // TILE          # 512-cell tile id
    pos = lcell - t * TILE     # position within tile (= matmul column)
    pair = t // 2              # tile 2j pairs with tile 2j+1
    blk = t % 2                # channel block within the pair

    key = core * NP + pair
    order = np.argsort(key, kind="stable")
    ks = key[order]
    counts = np.bincount(ks, minlength=NCORE * NP)
    maxcnt = counts.reshape(NCORE, NP).max(axis=0)
    chunks = np.maximum((maxcnt + CAP - 1) // CAP, 1)
    kpair = np.maximum((maxcnt + chunks - 1) // chunks, 1)
    ncols = int(chunks.sum())
    colbase = np.concatenate([[0], np.cumsum(chunks)]).astype(np.int64)

    kg = np.zeros(NGRP, np.int64)
    for g in range(NGRP):
        p0, p1 = g * SG, min((g + 1) * SG, NP)
        kg[g] = int(kpair[p0:p1].max())
        need = int(chunks[p0:p1].sum())
        if need > FB:
            raise ValueError("pair group needs %d cols > FB=%d" % (need, FB))

    starts = np.concatenate([[0], np.cumsum(counts)]).astype(np.int64)
    rank = np.arange(len(ks), dtype=np.int64) - starts[ks]

    co = core[order]
    po = pair[order]
    bo = blk[order]
    kp = kpair[po]
    colo = colbase[po] + rank // kp
    slot = rank % kp

    feat = np.zeros((NCORE, CAP, ncols * 128), np.float16)
    ccol = (colo * 128 + bo * C)[:, None] + np.arange(C)[None, :]
    feat[co[:, None], slot[:, None], ccol] = vfv[order]

    iota = np.broadcast_to(
        np.arange(TILE, dtype=np.float16)[None, :], (CAP, TILE)).copy()
    post = np.full((NCORE, CAP, ncols), -1.0, np.float32)
    post[co, slot, colo] = pos[order].astype(np.float32)

    maps = []
    for core_id in range(NCORE):
        bb, hh = core_id // 2, core_id % 2
        m = mf[bb, :, hh * NYH:(hh + 1) * NYH, :]      # [NX, NYH, CM]
        maps.append(np.ascontiguousarray(
            np.transpose(m, (2, 1, 0)).astype(np.float16)).reshape(CM, NCELL))
    return feat, iota, post, maps, ncols, chunks, kg


def kernel(voxel_features, coords, batch_size=None, map_fm=None,
           trace=False, _return_results=False):
    from concourse.bass_utils import run_bass_kernel_spmd

    feat, iota, post, maps, ncols, chunks, kg = _host_prep(
        voxel_features, coords, map_fm)

    ckey = (ncols, tuple(int(c) for c in chunks), tuple(int(k) for k in kg))
    if ckey not in _prog_cache:
        _prog_cache.clear()
        _prog_cache[ckey] = _build_program(ncols, chunks, kg)
    nc = _prog_cache[ckey]

    in_maps = [
        {"feat": feat[i], "iota": iota, "post": post[i], "mapin": maps[i]}
        for i in range(NCORE)
    ]
    res = run_bass_kernel_spmd(nc, in_maps, list(range(NCORE)), trace=trace)

    out = np.empty((B, C + CM, NY, NX), np.float32)
    for core_id in range(NCORE):
        bb, hh = core_id // 2, core_id % 2
        blk = np.empty((C + CM, NCELL), np.float32)
        # de-interleave tile halves: psum row blk*64+ch, col pair*512+pos
        can = res.results[core_id]["out"].reshape(2, C, NP, TILE)
        blk[:C] = (np.transpose(can, (1, 2, 0, 3))
                   .reshape(C, NP * 2 * TILE)[:, :NCELL])
        blk[C:] = res.results[core_id]["outm"]
        out[bb, :, hh * NYH:(hh + 1) * NYH, :] = blk.reshape(C + CM, NYH, NX)
    if _return_results:
        return out, res
    return out


# revision 26
# speedup vs baseline: 1.2243x; 1.2243x over previous
"""PointPillarsScatter Trainium2 kernel (fp16, bandwidth-optimized).

Reference op:
  canvas[b*NY*NX + y*NX + x] = voxel_features[p]        (scatter-set, 64 ch)
  out[:, :64]  = canvas -> [B, 64, NY, NX]
  out[:, 64:]  = transpose(map_fm, (0, 3, 2, 1))        (16 ch)

Strategy (8 NeuronCores, SPMD), core = batch*2 + y_half:
  The op is pure data movement, so the kernel is sized by HBM traffic.
  Everything runs in fp16 (abs rel err ~2^-11, far inside the 2e-2
  gate): the 34MB/core of fp32 output becomes 17MB, and all input
  tables are fp16 too.

  Scatter = one-hot matmul on the TensorEngine:
    out[128ch', 512cells] = featT[K, 128ch'].T @ S[K, 512]
  with S[s, n] = (pos[s] == n) built by DVE is_equal against a
  DMA-loaded iota table.  ch' packs the 64 channels of TWO 512-cell
  tiles (tile j -> psum 0:64, tile j+105 -> 64:128) so one matmul
  covers 1024 cells and each partition half maps to contiguous DRAM.
  K (points per column) is the per-group max point count over cores,
  baked into the program, so the feat table DMA only reads the
  occupied slots (~2.2MB/core instead of a padded 12MB).

  map_fm is transposed to [CM, NYH, NX] fp16 on the host; the device
  moves it with a single DRAM->DRAM DMA on the (otherwise idle)
  gpsimd SWDGE queue.  Canvas A/B-half writes go on the sync/scalar
  HWDGE queues respectively, balancing all three DMA paths.

Host side only computes index tables + shards/reformats inputs (per
the sharding hint); all arithmetic that produces output values runs
on device.  The host up-casts the returned fp16 shards to fp32.
"""

import sys

for _p in ("/opt/trn_rl_repo",):
    if _p not in sys.path:
        sys.path.insert(0, _p)

import numpy as np

# problem constants (hardcoded per contract)
B, NPTS, C, NY, NX, CM = 4, 48000, 64, 496, 432, 16
NYH = NY // 2            # 248 rows per core
NCORE = 8
NCELL = NYH * NX         # 107136 cells per core
TILE = 512               # cells per channel-block
NT = (NCELL + TILE - 1) // TILE          # 210 tiles (last has 128 cells)
NP = NT // 2                             # 105 pairs: tile 2j with 2j+1
LAST = NCELL - (NT - 1) * TILE           # 128 cells in the last tile
CAP = 128                # max points per column (K dim of the matmul)
SG = 15                  # pairs per group -> 7 groups, ~2MB out DMAs
NGRP = (NP + SG - 1) // SG
FB = 18                  # column budget of the per-group feat tile

_prog_cache = {}


def _build_program(ncols, chunks, kg):
    """Build the SPMD Bass program (identical for all 8 cores)."""
    from concourse import bacc, mybir, tile

    f16 = mybir.dt.float16
    f32 = mybir.dt.float32

    nc = bacc.Bacc(trn_type="TRN2", target_bir_lowering=False)

    feat_d = nc.dram_tensor("feat", [CAP, ncols * 128], f16,
                            kind="ExternalInput")
    iota_d = nc.dram_tensor("iota", [CAP, TILE], f16, kind="ExternalInput")
    post_d = nc.dram_tensor("post", [CAP, ncols], f32, kind="ExternalInput")
    map_d = nc.dram_tensor("mapin", [CM, NCELL], f16, kind="ExternalInput")
    # canvas in psum layout: row = blk*64+ch, col = pair*512+pos; the
    # host de-interleaves tile halves during the gather step
    out_d = nc.dram_tensor("out", [128, NP * TILE], f16,
                           kind="ExternalOutput")
    outm_d = nc.dram_tensor("outm", [CM, NCELL], f16, kind="ExternalOutput")

    colbase = np.concatenate([[0], np.cumsum(chunks)]).astype(np.int64)

    with tile.TileContext(nc) as tc:
        with (
            tc.tile_pool(name="const", bufs=1) as cpool,
            tc.tile_pool(name="fpool", bufs=3) as fpool,
            tc.tile_pool(name="spool", bufs=4) as spool,
            tc.tile_pool(name="stg", bufs=3) as stpool,
            tc.tile_pool(name="pscat", bufs=3, space="PSUM") as pspool,
        ):
            # DMA-loaded iota table + per-column slot positions
            iota_f = cpool.tile([CAP, TILE], f16)
            nc.sync.dma_start(out=iota_f[:], in_=iota_d[:])
            posT = cpool.tile([CAP, ncols], f32)
            nc.sync.dma_start(out=posT[:], in_=post_d[:])

            # map passthrough: DRAM->DRAM chunks, one per group, so the
            # map bandwidth fills the gaps of the scatter pipeline
            # instead of starving its head.
            mchunk = (NCELL + NGRP - 1) // NGRP

            def emit_map_chunk(g, eng):
                m0 = g * mchunk
                m1 = min(m0 + mchunk, NCELL)
                eng.dma_start(out=outm_d[:, m0:m1],
                              in_=map_d[:, m0:m1])

            def emit_feat_load(g):
                p0, p1 = g * SG, min((g + 1) * SG, NP)
                c0, c1 = int(colbase[p0]), int(colbase[p1])
                K = int(kg[g])
                fb = fpool.tile([CAP, FB * 128], f16, tag="fb")
                # sync queue, emitted before older groups' out DMAs so
                # the in-order sync engine issues the prefetch first.
                # Full 128 partitions: partial-partition DMAs serialize
                # onto a single SDMA engine (~27GB/s).
                nc.sync.dma_start(out=fb[:, :(c1 - c0) * 128],
                                  in_=feat_d[:, c0 * 128:c1 * 128])
                return fb

            fbs = {0: emit_feat_load(0), 1: emit_feat_load(1)}
            for g in range(NGRP):
                p0 = g * SG
                p1 = min(p0 + SG, NP)
                c0 = int(colbase[p0])
                K = int(kg[g])
                fb = fbs.pop(g)
                if g + 2 < NGRP:
                    fbs[g + 2] = emit_feat_load(g + 2)
                emit_map_chunk(g, nc.scalar)
                stg = stpool.tile([128, SG * TILE], f16, tag="stg")
                # pairs are processed two-at-a-time into a 2-bank psum
                # tile; the copy drains both pairs at once and is emitted
                # AFTER the next pair's mask so DVE keeps the PE fed.
                pend = None        # (psum tile, stg offset, width)
                for pr in range(p0, p1):
                    half = (pr - p0) % 2
                    if half == 0:
                        npair = min(2, p1 - pr)
                        ps = pspool.tile([128, npair * TILE], f32)
                    nck = int(chunks[pr])
                    for k in range(nck):
                        col = int(colbase[pr]) + k
                        s_t = spool.tile([CAP, TILE], f16)
                        nc.vector.tensor_scalar(
                            out=s_t[:K], in0=iota_f[:K, :],
                            scalar1=posT[:K, col:col + 1],
                            scalar2=None,
                            op0=mybir.AluOpType.is_equal)
                        lhs = fb[:K, (col - c0) * 128:(col - c0 + 1) * 128]
                        nc.tensor.matmul(
                            out=ps[:, half * TILE:(half + 1) * TILE],
                            lhsT=lhs, rhs=s_t[:K],
                            start=(k == 0), stop=(k == nck - 1))
                    if pend is not None:
                        cps, coff, cw = pend
                        pend = None
                        if (pr // 2) % 4 == 0:
                            nc.vector.tensor_copy(
                                out=stg[:, coff:coff + cw], in_=cps[:, :cw])
                        else:
                            nc.scalar.copy(
                                out=stg[:, coff:coff + cw], in_=cps[:, :cw])
                    if half == npair - 1:
                        pend = (ps, (pr - p0 - half) * TILE, npair * TILE)
                if pend is not None:
                    cps, coff, cw = pend
                    nc.scalar.copy(out=stg[:, coff:coff + cw],
                                   in_=cps[:, :cw])
                # full-width out DMA in psum layout; split for the last
                # group so the final drain overlaps the last copies
                oeng = nc.sync if g % 2 == 0 else nc.scalar
                wa = (p1 - p0) * TILE
                if g == NGRP - 1:
                    wh = (wa // (2 * TILE)) * TILE
                    oeng.dma_start(out=out_d[:, p0 * TILE:p0 * TILE + wh],
                                   in_=stg[:, :wh])
                    oeng.dma_start(
                        out=out_d[:, p0 * TILE + wh:p0 * TILE + wa],
                        in_=stg[:, wh:wa])
                else:
                    oeng.dma_start(out=out_d[:, p0 * TILE:p0 * TILE + wa],
                                   in_=stg[:, :wa])

    nc.finalize()
    return nc


def _host_prep(voxel_features, coords, map_fm):
    """Shard points by core, build feat/pos tables (host index work only)."""
    vf = np.asarray(voxel_features).astype(np.float16)
    cd = np.asarray(coords)
    mf = np.asarray(map_fm)
    if mf.ndim == 5:
        mf = np.squeeze(mf, 3)

    b = cd[:, 0].astype(np.int64)
    y = cd[:, 2].astype(np.int64)
    x = cd[:, 3].astype(np.int64)
    valid = (b >= 0) & (b < B) & (y >= 0) & (y < NY) & (x >= 0) & (x < NX)
    b, y, x = b[valid], y[valid], x[valid]
    vfv = vf[valid]

    half = (y >= NYH).astype(np.int64)
    core = b * 2 + half
    lcell = (y - half * NYH) * NX + x
    t = lcell // TILE          # 512-cell tile id
    pos = lcell - t * TILE     # position within tile (= matmul column)
    pair = t // 2              # tile 2j pairs with tile 2j+1
    blk = t % 2                # channel block within the pair

    key = core * NP + pair
    order = np.argsort(key, kind="stable")
    ks = key[order]
    counts = np.bincount(ks, minlength=NCORE * NP)
    maxcnt = counts.reshape(NCORE, NP).max(axis=0)
    chunks = np.maximum((maxcnt + CAP - 1) // CAP, 1)
    kpair = np.maximum((maxcnt + chunks - 1) // chunks, 1)
    ncols = int(chunks.sum())
    colbase = np.concatenate([[0], np.cumsum(chunks)]).astype(np.int64)

    kg = np.zeros(NGRP, np.int64)
    for g in range(NGRP):
        p0, p1 = g * SG, min((g + 1) * SG, NP)
        kg[g] = int(kpair[p0:p1].max())
        need = int(chunks[p0:p1].sum())
        if need > FB:
            raise ValueError("pair group needs %d cols > FB=%d" % (need, FB))

    starts = np.concatenate([[0], np.cumsum(counts)]).astype(np.int64)
    rank = np.arange(len(ks), dtype=np.int64) - starts[ks]

    co = core[order]
    po = pair[order]
    bo = blk[order]
    kp = kpair[po]
    colo = colbase[po] + rank // kp
    slot = rank % kp

    feat = np.zeros((NCORE, CAP, ncols * 128), np.float16)
    ccol = (colo * 128 + bo * C)[:, None] + np.arange(C)[None, :]
    feat[co[:, None], slot[:, None], ccol] = vfv[order]

    iota = np.broadcast_to(
        np.arange(TILE, dtype=np.float16)[None, :], (CAP, TILE)).copy()
    post = np.full((NCORE, CAP, ncols), -1.0, np.float32)
    post[co, slot, colo] = pos[order].astype(np.float32)

    maps = []
    for core_id in range(NCORE):
        bb, hh = core_id // 2, core_id % 2
        m = mf[bb, :, hh * NYH:(hh + 1) * NYH, :]      # [NX, NYH, CM]
        maps.append(np.ascontiguousarray(
            np.transpose(m, (2, 1, 0)).astype(np.float16)).reshape(CM, NCELL))
    return feat, iota, post, maps, ncols, chunks, kg


def kernel(voxel_features, coords, batch_size=None, map_fm=None,
           trace=False, _return_results=False):
    from concourse.bass_utils import run_bass_kernel_spmd

    feat, iota, post, maps, ncols, chunks, kg = _host_prep(
        voxel_features, coords, map_fm)

    ckey = (ncols, tuple(int(c) for c in chunks), tuple(int(k) for k in kg))
    if ckey not in _prog_cache:
        _prog_cache.clear()
        _prog_cache[ckey] = _build_program(ncols, chunks, kg)
    nc = _prog_cache[ckey]

    in_maps = [
        {"feat": feat[i], "iota": iota, "post": post[i], "mapin": maps[i]}
        for i in range(NCORE)
    ]
    res = run_bass_kernel_spmd(nc, in_maps, list(range(NCORE)), trace=trace)

    out = np.empty((B, C + CM, NY, NX), np.float32)
    for core_id in range(NCORE):
        bb, hh = core_id // 2, core_id % 2
        blk = np.empty((C + CM, NCELL), np.float32)
        # de-interleave tile halves: psum row blk*64+ch, col pair*512+pos
        can = res.results[core_id]["out"].reshape(2, C, NP, TILE)
        blk[:C] = (np.transpose(can, (1, 2, 0, 3))
                   .reshape(C, NP * 2 * TILE)[:, :NCELL])
        blk[C:] = res.results[core_id]["outm"]
        out[bb, :, hh * NYH:(hh + 1) * NYH, :] = blk.reshape(C + CM, NYH, NX)
    if _return_results:
        return out, res
    return out
